# revision 52
# baseline (speedup 1.0000x reference)
"""Trainium2 Bass kernel for BaselineDNN (ragged embedding pooling + MLP).

Data-parallel over batch (8 cores), 512 rows/core as 4 slots of 128.

The SWDGE Q7 descriptor generation (~4.8ns/desc, serial) is the critical
path, so the design minimizes *valid* gather descriptors per core:

  - rows globally sorted by length -> 32 blocks of 128 consecutive rows
    (length spread ~4 inside a block, vs ~32 for per-core sorting).
  - slot j holds global blocks 8j..8j+7, one per core, assigned greedily so
    every core's total wave count is nearly equal (load balance).
  - the shared SPMD schedule sizes each gather for the slot's max core;
    cores with shorter blocks skip their trailing waves at run time via a
    per-core `num_idxs_reg` register (trailing skip is the documented-safe
    dma_gather path).  Skipped waves are never fetched: no Q7 descriptor
    time, no HBM traffic.
  - gather tiles whose trailing waves may be skipped by some core are
    pre-memset to -1; the pad table row is also -1.  The resulting -1
    contributions to the sum pool are exactly corrected per row via the
    ACT bias (host knows every pad/stale count); -1 never wins the max.

Table: fp32 [50000, 300] repacked to fp16 [50004, 384] (768B rows, 256B
multiple): [pad_neg, pad_zero, emb..., pad_zero, pad_neg].  int16 gather
indices address two overlapping windows (lo = rows [0,32768), hi = rows
[17236, 50004)); each row's tokens are split to balance lo/hi counts.

Device pipeline per slot:
  - chained dma_gather (W<=8 waves) over 4 SWDGE queues, 12 tile buffers;
    non-ragged gathers use a static count (no register load); ragged
    gathers are scheduled early in the slot so their fills resolve away
    from the slot-end completion burst.
  - DVE: per-gather fp16 max accumulate; unit-stride TT-max tree per slot.
  - PE: per-wave identity matmuls accumulate the sum pool in f32 PSUM,
    then fp16 matmuls for the MLP.
  - ACT: avg = (psum + corr) * (1/len) straight from PSUM; relu; logits.
  - per-slot output DMA; emission interleaves the last gathers of each
    slot with the next slot's first gathers and spreads the previous
    slot's reduce/MLP between gathers, so the PE/DVE consumer backlog
    never stalls SWDGE semaphore-lane reuse at slot boundaries.
"""
import sys

sys.path.insert(0, "/opt/trn_rl_repo")

import numpy as np

import concourse.bacc as bacc
import concourse.bass as bass
import concourse.mybir as mybir
import concourse.tile as tile
from concourse.bass_utils import run_bass_kernel_spmd
from concourse.masks import make_identity

VOCAB, EMB_DIM, HIDDEN, NUM_CLASSES = 50000, 300, 1000, 5
B, MAX_LEN = 4096, 128
NCORES = 8
ROWS_PER_CORE = B // NCORES          # 512
NSLOTS = ROWS_PER_CORE // 128        # 4
NGB = B // 128                       # 32 global blocks
E_PAD = 384                          # fp16 row: 768B (256B multiple)
DEV_ROWS = VOCAB + 4
LO_SIZE = 32768
HI_BASE = DEV_ROWS - 32768           # 17236
PAD_NEG = -1.0
W_MAX = 8                            # waves per dma_gather
NQ = 4                               # SWDGE queues
GBUFS = 12                           # gather tile buffers in flight
OVERLAP = 12                         # cross-slot gather interleave window
KC, MC = 120, 125                    # matmul k-chunk / m-chunk
SLOT_ORDER = (3, 2, 1, 0)            # longest first, shortest last (tail)

_dt = mybir.dt


def _emission_order(sched):
    """Global gather emission order: slots in SLOT_ORDER, with the last
    OVERLAP gathers of each slot interleaved with the next slot's first
    OVERLAP gathers (cross-slot pipelining smooths the per-slot completion
    burst that otherwise stalls SWDGE semaphore-lane reuse)."""
    order = []
    prev_tail = []
    for j in SLOT_ORDER:
        n = len(sched[j])
        k = min(OVERLAP, n // 2)
        glist = [(j, gi) for gi in range(n)]
        head, mid, tail = glist[:k], glist[k : n - k], glist[n - k :]
        merged = []
        for i in range(max(len(prev_tail), len(head))):
            if i < len(prev_tail):
                merged.append(prev_tail[i])
            if i < len(head):
                merged.append(head[i])
        order += merged + mid
        prev_tail = tail
    order += prev_tail
    return order


def _plan(x, lengths):
    x = np.asarray(x)
    lengths = np.asarray(lengths).astype(np.int64)
    order = np.argsort(lengths, kind="stable")

    # per global block: per-row balanced lo/hi token lists
    gb_rows = [order[g * 128 : (g + 1) * 128] for g in range(NGB)]
    gb_lo = [[None] * 128 for _ in range(NGB)]
    gb_hi = [[None] * 128 for _ in range(NGB)]
    gb_wlo = np.zeros(NGB, np.int64)
    gb_whi = np.zeros(NGB, np.int64)
    for g in range(NGB):
        for p, r in enumerate(gb_rows[g]):
            d = x[r, : lengths[r]].astype(np.int64) + 2
            forced_lo = d[d < HI_BASE]
            forced_hi = d[d >= LO_SIZE]
            flex = d[(d >= HI_BASE) & (d < LO_SIZE)]
            t = len(d)
            lo_take = int(np.clip((t + 1) // 2 - len(forced_lo), 0, len(flex)))
            lo = np.concatenate([forced_lo, flex[:lo_take]])
            hi = np.concatenate([forced_hi, flex[lo_take:]])
            gb_lo[g][p] = lo.astype(np.int16)
            gb_hi[g][p] = (hi - HI_BASE).astype(np.int16)
        gb_wlo[g] = max(len(v) for v in gb_lo[g])
        gb_whi[g] = max(len(v) for v in gb_hi[g])

    # slot j = global blocks 8j..8j+7; greedy core assignment balances totals
    gb_w = gb_wlo + gb_whi
    assign = np.zeros((NCORES, NSLOTS), np.int64)   # core, slot -> global block
    totals = np.zeros(NCORES, np.int64)
    for j in range(NSLOTS - 1, -1, -1):             # big slots first
        blocks = sorted(range(8 * j, 8 * j + 8), key=lambda g: -gb_w[g])
        for g in blocks:
            c = int(np.argmin(totals))
            assign[c, j] = g
            totals[c] += gb_w[g]

    # shared schedule per slot: (w, col_off, is_lo); max wave count first
    sched = {}
    slot_cols = {}
    slot_wlo = {}
    slot_whi = {}
    for j in range(NSLOTS):
        Wlo = int(max(gb_wlo[assign[c, j]] for c in range(NCORES)))
        Whi = int(max(gb_whi[assign[c, j]] for c in range(NCORES)))
        slot_wlo[j], slot_whi[j] = Wlo, Whi
        s = []
        col = 0
        for is_lo, C in ((True, Wlo), (False, Whi)):
            w0 = 0
            while w0 < C:
                w = min(W_MAX, C - w0)
                s.append([w, col, is_lo, w0])
                col += w * 8
                w0 += w
        s.sort(key=lambda t: -t[0])
        sched[j] = s
        slot_cols[j] = col

    idx_arrs = {j: np.full((NCORES, 128, slot_cols[j]), -1, np.int16)
                for j in range(NSLOTS)}
    # valid wave count per (core, slot, gather)
    valid = {j: np.zeros((NCORES, len(sched[j])), np.int32) for j in range(NSLOTS)}
    scale = np.zeros((NCORES, 128, NSLOTS), np.float32)
    bias = np.zeros((NCORES, 128, NSLOTS), np.float32)
    for c in range(NCORES):
        for j in range(NSLOTS):
            g = assign[c, j]
            Wlo_c, Whi_c = int(gb_wlo[g]), int(gb_whi[g])
            # wave matrices padded to the core's own block maxes (pad row:
            # lo window row 0 / hi window row 32767, both PAD_NEG)
            wl = np.full((Wlo_c, 128), 0, np.int16)
            wh = np.full((Whi_c, 128), 32767, np.int16)
            for p in range(128):
                lo, hi = gb_lo[g][p], gb_hi[g][p]
                wl[: len(lo), p] = lo
                wh[: len(hi), p] = hi
            ln = lengths[gb_rows[g]].astype(np.float32)
            n_stale = 0
            n_forced = 0
            for gi, (w, off, is_lo, w0) in enumerate(sched[j]):
                Wc = Wlo_c if is_lo else Whi_c
                mat = wl if is_lo else wh
                v = int(np.clip(Wc - w0, 1, w))
                valid[j][c, gi] = v
                n_stale += w - v
                n_forced += v - max(0, min(v, Wc - w0))
                span = max(0, min(v, Wc - w0))
                sl = mat[w0 : w0 + span]
                if span < v:
                    sl = np.concatenate(
                        [sl, np.full((v - span, 128), 0 if is_lo else 32767,
                                     np.int16)])
                flat = sl.reshape(-1)
                wrapped = flat.reshape(-1, 16).T
                idx_arrs[j][c, :, off : off + v * 8] = np.tile(wrapped, (8, 1))
            npad = (Wlo_c + Whi_c) - ln          # pad-row tokens inside valid waves
            scale[c, :, j] = 1.0 / ln
            # pad-row tokens and forced pad waves contributed PAD_NEG = -1
            # (stale-consumed waves are added below once fills are known)
            bias[c, :, j] = (npad + n_forced) / ln
    # per-gather fill window [vmin, vmax); only waves [0, vmax) are consumed
    fills = {}
    for j in range(NSLOTS):
        v = valid[j]
        fills[j] = [(int(v[:, gi].min()), int(v[:, gi].max()))
                    for gi in range(v.shape[1])]
        # move ragged gathers (which need DMA fills) right after the first
        # gather: their fills then resolve early in the slot, far from the
        # slot-end completion burst that stalls SWDGE lane reuse
        n = len(sched[j])
        perm = [0] + [gi for gi in range(1, n) if fills[j][gi][0] < fills[j][gi][1]] \
                   + [gi for gi in range(1, n) if fills[j][gi][0] >= fills[j][gi][1]]
        sched[j] = [sched[j][gi] for gi in perm]
        fills[j] = [fills[j][gi] for gi in perm]
        valid[j] = valid[j][:, perm]
        v = valid[j]
        if j == 0:
            # slot-0 stale waves are filled with -1 (short rows; 0 would be
            # max-unsafe); each consumed stale wave [v, vmax) added -1
            for c in range(NCORES):
                n_cons = sum(max(0, fills[j][gi][1] - int(v[c, gi]))
                             for gi in range(v.shape[1]))
                ln = lengths[gb_rows[assign[c, j]]].astype(np.float32)
                bias[c, :, j] += n_cons / ln
    rows_by_core = np.stack([
        np.concatenate([gb_rows[assign[c, j]] for j in range(NSLOTS)])
        for c in range(NCORES)])          # [NCORES, 512] global row ids
    inv_perm = np.empty(B, np.int64)
    inv_perm[rows_by_core.reshape(-1)] = np.arange(B)
    return dict(sched=sched, slot_cols=slot_cols, idx=idx_arrs, valid=valid,
                scale=scale, bias=bias, inv_perm=inv_perm, fills=fills)


def _build_nc(sched, slot_cols, fills, n_gathers):
    nc = bacc.Bacc("TRN2", target_bir_lowering=False, debug=False,
                   num_swdge_queues=NQ)
    table = nc.declare_dram_parameter("table", [DEV_ROWS, E_PAD], _dt.float16, isOutput=False)
    idx_d = {j: nc.declare_dram_parameter(f"idx{j}", [128, slot_cols[j]], _dt.int16,
                                          isOutput=False) for j in range(NSLOTS)}
    cnt_d = nc.declare_dram_parameter("cnt", [1, n_gathers], _dt.int32, isOutput=False)
    filln_d = nc.declare_dram_parameter("filln", [128, W_MAX * E_PAD], _dt.float16,
                                        isOutput=False)
    fillz_d = nc.declare_dram_parameter("fillz", [128, W_MAX * E_PAD], _dt.float16,
                                        isOutput=False)
    sb = nc.declare_dram_parameter("sb", [128, 2 * NSLOTS], _dt.float32, isOutput=False)
    w1 = nc.declare_dram_parameter("w1", [2 * EMB_DIM, HIDDEN], _dt.float16, isOutput=False)
    b1 = nc.declare_dram_parameter("b1", [HIDDEN], _dt.float32, isOutput=False)
    w2 = nc.declare_dram_parameter("w2", [HIDDEN, NUM_CLASSES], _dt.float16, isOutput=False)
    b2 = nc.declare_dram_parameter("b2", [NUM_CLASSES], _dt.float32, isOutput=False)
    out = nc.declare_dram_parameter("out", [ROWS_PER_CORE, NUM_CLASSES], _dt.float32, isOutput=True)

    table_lo = table[0:LO_SIZE, :]
    table_hi = table[HI_BASE:DEV_ROWS, :]

    qctr = [0]

    def next_q():
        q = qctr[0] % NQ
        qctr[0] += 1
        return q

    regs = [nc.gpsimd.alloc_register(f"nidx{i}") for i in range(8)]
    rctr = [0]

    def next_reg():
        r = regs[rctr[0] % len(regs)]
        rctr[0] += 1
        return r

    with tile.TileContext(nc) as tc:
        with (
            tc.tile_pool(name="const", bufs=1) as cpool,
            tc.tile_pool(name="gather", bufs=GBUFS) as gpool,
            tc.tile_pool(name="acc", bufs=2) as apool,
            tc.tile_pool(name="red", bufs=2) as rpool,
            tc.tile_pool(name="mlp", bufs=2) as mpool,
            tc.tile_pool(name="psum", bufs=2, space="PSUM") as ppool,
            tc.tile_pool(name="psum2", bufs=2, space="PSUM") as ppool2,
            tc.tile_pool(name="psums", bufs=2, space="PSUM") as ppool3,
        ):
            # counts first (tiny, gates the first reg_load), then per-slot idx
            cnt_t = cpool.tile([1, n_gathers], _dt.int32)
            nc.sync.dma_start(out=cnt_t[:], in_=cnt_d[:])
            idx_t = {}
            for j in SLOT_ORDER:
                t = cpool.tile([128, slot_cols[j]], _dt.int16, tag=f"idx{j}")
                nc.sync.dma_start(out=t[:], in_=idx_d[j][:])
                idx_t[j] = t
            sb_t = cpool.tile([128, 2 * NSLOTS], _dt.float32)
            nc.sync.dma_start(out=sb_t[:], in_=sb[:])
            w1_t = cpool.tile([KC, 5 * HIDDEN], _dt.float16)
            for k in range(5):
                nc.sync.dma_start(out=w1_t[:, k * HIDDEN : (k + 1) * HIDDEN],
                                  in_=w1[k * KC : (k + 1) * KC, :])
            b1_t = cpool.tile([MC, 8], _dt.float32)
            nc.sync.dma_start(out=b1_t[:], in_=b1[:].rearrange("(m p) -> p m", p=MC))
            w2_t = cpool.tile([MC, 8 * NUM_CLASSES], _dt.float16)
            for m in range(8):
                nc.sync.dma_start(out=w2_t[:, m * NUM_CLASSES : (m + 1) * NUM_CLASSES],
                                  in_=w2[m * MC : (m + 1) * MC, :])
            b2_t = cpool.tile([NUM_CLASSES, 1], _dt.float32)
            nc.sync.dma_start(out=b2_t[:], in_=b2[:, None])
            ident = cpool.tile([128, 128], _dt.float16)
            make_identity(nc, ident[:])

            def finish_thunks(j, max_acc, psum_sum, wa):
                """Reduce/avg/MLP/out for slot j as a list of small emission
                chunks, interleaved into the next slot's gather stream so the
                PE queue never has a long burst blocking gather-tile reuse."""
                rep = rpool.tile([128, 2 * EMB_DIM], _dt.float16, tag="rep")
                repT = mpool.tile([KC, 5 * 128], _dt.float16, tag="repT")
                hT = mpool.tile([MC, 8 * 128], _dt.float16, tag="hT")
                thunks = []

                def t_reduce():
                    # unit-stride TT-max tree over the wave axis (overlapping
                    # halves are harmless for max); much cheaper on DVE than
                    # a strided reduce_max
                    width = wa
                    if width == 1:
                        nc.vector.tensor_copy(out=rep[:, EMB_DIM : 2 * EMB_DIM],
                                              in_=max_acc[:, 0, 0:EMB_DIM])
                    while width > 1:
                        h = (width + 1) // 2
                        if width > 2:
                            nc.vector.tensor_tensor(
                                out=max_acc[:, 0:h, :],
                                in0=max_acc[:, 0:h, :],
                                in1=max_acc[:, width - h : width, :],
                                op=mybir.AluOpType.max)
                        else:
                            nc.vector.tensor_tensor(
                                out=rep[:, EMB_DIM : 2 * EMB_DIM],
                                in0=max_acc[:, 0, 0:EMB_DIM],
                                in1=max_acc[:, width - 1, 0:EMB_DIM],
                                op=mybir.AluOpType.max)
                        width = h
                    nc.scalar.activation(
                        rep[:, 0:EMB_DIM], psum_sum[:],
                        mybir.ActivationFunctionType.Identity,
                        bias=sb_t[:, NSLOTS + j : NSLOTS + j + 1],
                        scale=sb_t[:, j : j + 1],
                    )
                thunks.append(t_reduce)

                def t_transpose(k):
                    tp = ppool.tile([KC, 128], _dt.float16, tag="tp", space="PSUM")
                    nc.tensor.transpose(out=tp[:], in_=rep[:, k * KC : (k + 1) * KC],
                                        identity=ident[:])
                    nc.vector.tensor_copy(out=repT[:, k * 128 : (k + 1) * 128],
                                          in_=tp[:])
                for k in range(5):
                    thunks.append(lambda k=k: t_transpose(k))

                def t_w1(m):
                    hp = ppool.tile([MC, 128], _dt.float32, tag="hp", space="PSUM")
                    for k in range(5):
                        nc.tensor.matmul(
                            hp[:],
                            w1_t[:, k * HIDDEN + m * MC : k * HIDDEN + (m + 1) * MC],
                            repT[:, k * 128 : (k + 1) * 128],
                            start=(k == 0), stop=(k == 4),
                        )
                    nc.scalar.activation(
                        hT[:, m * 128 : (m + 1) * 128], hp[:],
                        mybir.ActivationFunctionType.Relu,
                        bias=b1_t[:, m : m + 1],
                    )
                for m in range(8):
                    thunks.append(lambda m=m: t_w1(m))

                def t_w2():
                    lp = ppool2.tile([NUM_CLASSES, 128], _dt.float32, tag="lp",
                                     space="PSUM")
                    for m in range(8):
                        nc.tensor.matmul(
                            lp[:],
                            w2_t[:, m * NUM_CLASSES : (m + 1) * NUM_CLASSES],
                            hT[:, m * 128 : (m + 1) * 128],
                            start=(m == 0), stop=(m == 7),
                        )
                    logitsT = rpool.tile([NUM_CLASSES, 128], _dt.float32, tag="lg")
                    nc.scalar.activation(
                        logitsT[:], lp[:],
                        mybir.ActivationFunctionType.Identity,
                        bias=b2_t[:, 0:1],
                    )
                    nc.sync.dma_start(
                        out=out[j * 128 : (j + 1) * 128, :].rearrange("r c -> c r"),
                        in_=logitsT[:])
                thunks.append(t_w2)
                return thunks

            order = _emission_order(sched)
            gidx = 0
            pending = []
            state = {}
            for (j, gi) in order:
                if gi == 0:
                    state[j] = dict(
                        max_acc=apool.tile([128, W_MAX, E_PAD], _dt.float16,
                                           tag="macc", name=f"macc{j}"),
                        psum_sum=ppool3.tile([128, EMB_DIM], _dt.float32,
                                             tag="ps", space="PSUM",
                                             name=f"ps{j}"),
                        wa=max(fills[j][g][1] for g in range(len(sched[j]))),
                    )
                st = state[j]
                max_acc, psum_sum = st["max_acc"], st["psum_sum"]
                n_g = len(sched[j])
                w, off, is_lo, _w0 = sched[j][gi]
                vmin, vmax = fills[j][gi]
                wc = vmax          # consumed waves
                g_t = gpool.tile([128, W_MAX, E_PAD], _dt.float16, tag="g")
                if vmax > vmin:
                    # pre-fill the ragged window from a DRAM constant; use
                    # the scalar engine's HWDGE ring so a blocked fill can't
                    # head-of-line block the sync ring (idx/out DMAs)
                    fill_src = filln_d if j == 0 else fillz_d
                    nc.scalar.dma_start(
                        out=g_t[:, vmin:vmax, :].rearrange("p w e -> p (w e)"),
                        in_=fill_src[:, : (vmax - vmin) * E_PAD])
                src = table_lo if is_lo else table_hi
                if vmin == vmax:
                    # every core gathers the same wave count: no register
                    nidx_reg = vmin * 128
                else:
                    nidx_reg = next_reg()
                    nc.gpsimd.reg_load(nidx_reg, cnt_t[0:1, gidx : gidx + 1])
                nc.gpsimd.dma_gather(
                    g_t[:, :wc, :], src, idx_t[j][:, off : off + w * 8],
                    w * 128, nidx_reg, E_PAD, single_packet=False,
                    queue_num=next_q(),
                )
                gidx += 1
                if gi == 0:
                    nc.vector.tensor_copy(out=max_acc[:, :wc, :], in_=g_t[:, :wc, :])
                else:
                    nc.vector.tensor_tensor(
                        out=max_acc[:, :wc, 0:EMB_DIM],
                        in0=max_acc[:, :wc, 0:EMB_DIM],
                        in1=g_t[:, :wc, 0:EMB_DIM],
                        op=mybir.AluOpType.max)
                for wv in range(wc):
                    nc.tensor.matmul(
                        psum_sum[:],
                        ident[:],
                        g_t[:, wv, 0:EMB_DIM],
                        start=(gi == 0 and wv == 0),
                        stop=(gi == n_g - 1 and wv == wc - 1),
                    )
                # interleave a couple of the previous slot's finish chunks
                for _ in range(2):
                    if pending:
                        pending.pop(0)()
                if gi == n_g - 1:
                    while pending:
                        pending.pop(0)()
                    pending = finish_thunks(j, max_acc, psum_sum, st["wa"])
            while pending:
                pending.pop(0)()
    nc.compile()
    return nc


def kernel(x, lengths, emb_table, W1, b1, W2, b2, _trace=False, _trace_cores=None):
    x = np.asarray(x)
    lengths = np.asarray(lengths)
    plan = _plan(x, lengths)
    n_gathers = sum(len(plan["sched"][j]) for j in range(NSLOTS))
    nc = _build_nc(plan["sched"], plan["slot_cols"], plan["fills"], n_gathers)

    table_dev = np.zeros((DEV_ROWS, E_PAD), np.float16)
    table_dev[0, :] = PAD_NEG
    table_dev[-1, :] = PAD_NEG
    table_dev[2 : VOCAB + 2, :EMB_DIM] = np.asarray(emb_table, np.float32).astype(np.float16)

    in_maps = []
    for c in range(NCORES):
        sbv = np.concatenate([plan["scale"][c], plan["bias"][c]], axis=1).astype(np.float32)
        cnt = np.array([plan["valid"][j][c, gi]
                        for (j, gi) in _emission_order(plan["sched"])],
                       np.int32) * 128
        im = {
            "table": table_dev,
            "cnt": cnt[None, :],
            "filln": np.full((128, W_MAX * E_PAD), PAD_NEG, np.float16),
            "fillz": np.zeros((128, W_MAX * E_PAD), np.float16),
            "sb": sbv,
            "w1": np.asarray(W1, np.float32).astype(np.float16),
            "b1": np.asarray(b1, np.float32),
            "w2": np.asarray(W2, np.float32).astype(np.float16),
            "b2": np.asarray(b2, np.float32),
        }
        for j in range(NSLOTS):
            im[f"idx{j}"] = np.ascontiguousarray(plan["idx"][j][c])
        in_maps.append(im)
    kw = {}
    if _trace:
        kw = dict(trace=True, trace_cores=_trace_cores or [0])
    res = run_bass_kernel_spmd(nc, in_maps, core_ids=list(range(NCORES)), **kw)
    # out rows per core: slot-major (slot 0..3), matching rows_by_core
    logits_sorted = np.concatenate([res.results[c]["out"] for c in range(NCORES)], axis=0)
    logits = logits_sorted[plan["inv_perm"]]
    if _trace:
        return logits, res
    return logits
